# revision 17
# baseline (speedup 1.0000x reference)
"""ChebConv (K=3) message-passing kernel for 8 Trainium2 NeuronCores.

Strategy (sharding_hint: partition nodes / edges by dst across cores):
  - Core c owns dst-node rows [c*N/8, (c+1)*N/8).
  - Per propagate round, each core dma_gather's the messages h[src] for its
    edges from a replicated DRAM table (bf16), computes the scatter-mean via
    one-hot matrices multiplied on the tensor engine (PSUM accumulation per
    128-dst block), applies the Chebyshev recurrence, and contributes the
    term's part of the final linear layer.
  - Between rounds the new term's node table is AllGather'd (bf16) so every
    core can gather arbitrary src rows next round.
All per-core data is prepared host-side from the actual inputs; the single
SPMD program shape is made core-independent by padding each (block, src-half)
edge list to the max chunk count over cores.
"""

import numpy as np
import ml_dtypes

BF16 = ml_dtypes.bfloat16
P = 128  # partitions / block size
LO_LIMIT = 32768  # int16 index limit for dma_gather


# ---------------------------------------------------------------- host prep

def _prep(x, edge_index, W, b, n_cores):
    N, D = x.shape
    assert D == P and N % n_cores == 0
    npc = N // n_cores           # nodes per core
    nb = (npc + P - 1) // P      # dst blocks per core
    split_b = (nb + 1) // 2      # shard-half split (block-aligned)
    rows1 = min(split_b * P, npc)
    rows2 = npc - rows1
    src = np.asarray(edge_index[0]).astype(np.int64)
    dst = np.asarray(edge_index[1]).astype(np.int64)
    n_halves = 2

    cnt = np.bincount(dst, minlength=N).astype(np.float32)
    recip1 = (1.0 / np.maximum(cnt, 1.0)).astype(np.float32)

    # src node -> (table half, row within half-table); half tables hold each
    # core's first rows1 rows (half 0) / last rows2 rows (half 1), so the
    # AllGather of half 0 can fire mid-round and both tables stay < 32768 rows
    c_src = src // npc
    r_src = src % npc
    half_src = (r_src >= rows1).astype(np.int64)
    idx_src = np.where(half_src == 0, c_src * rows1 + r_src,
                       c_src * rows2 + (r_src - rows1)).astype(np.int64)
    assert n_cores * rows1 < 32768 and n_cores * rows2 < 32768

    core_of = dst // npc
    per_core = []
    counts = np.zeros((n_cores, nb, 2), np.int64)
    for c in range(n_cores):
        sel = core_of == c
        s_c = idx_src[sel]
        half = half_src[sel]
        d_c = dst[sel] - c * npc
        blk = d_c >> 7
        order = np.lexsort((s_c, half, blk))
        s_c, d_c, blk, half = s_c[order], d_c[order], blk[order], half[order]
        key = blk * 2 + half
        nbh = np.bincount(key, minlength=nb * 2).reshape(nb, 2)
        counts[c] = nbh
        per_core.append((s_c, d_c, key))

    # shared chunk capacities per (block, half)
    ch = -(-counts // P)                      # ceil div
    cap = ch.max(axis=0)                      # [nb, 2]
    for bidx in range(nb):
        if cap[bidx].sum() == 0:
            cap[bidx, 0] = 1                  # guarantee >=1 chunk per block

    # group blocks so a group's chunks fit the SBUF message buffer
    CMAX = 104
    groups, cur, cur_n = [], [], 0
    for bidx in range(nb):
        nchunks = int(cap[bidx].sum())
        if cur and cur_n + nchunks > CMAX:
            groups.append(cur)
            cur, cur_n = [], 0
        cur.append(bidx)
        cur_n += nchunks
    if cur:
        groups.append(cur)

    # global chunk-column layout: per group, lo chunks of its blocks then hi
    lo_col = np.zeros(nb, np.int64)
    hi_col = np.zeros(nb, np.int64)
    gmeta = []  # (col_start, n_lo_chunks, n_hi_chunks, block list)
    col = 0
    for blocks in groups:
        gstart = col
        for bidx in blocks:
            lo_col[bidx] = col
            col += cap[bidx, 0]
        glo = col - gstart
        for bidx in blocks:
            hi_col[bidx] = col
            col += cap[bidx, 1]
        ghi = col - gstart - glo
        gmeta.append((gstart, glo, ghi, blocks))
    ctot = col

    # per-core slot arrays
    idx_alls, dstrel_alls = [], []
    for c in range(n_cores):
        s_c, d_c, key = per_core[c]
        idx_slots = np.zeros(ctot * P, np.int16)
        dst_slots = np.full(ctot * P, -1, np.int16)
        nbh = counts[c].reshape(-1)
        # slot of each edge: base slot of its (block, half) + rank within it
        base = np.zeros(nb * 2, np.int64)
        base[0::2] = lo_col * P
        base[1::2] = hi_col * P
        starts = np.zeros(nb * 2, np.int64)
        starts[1:] = np.cumsum(nbh)[:-1]
        ranks = np.arange(len(s_c)) - starts[key]
        slots = base[key] + ranks
        idx_slots[slots] = s_c.astype(np.int16)
        dst_slots[slots] = (d_c - (d_c >> 7 << 7)).astype(np.int16)
        # wrapped index layout per gather call: [16, n/16] tiled to [128, .]
        parts = []
        for (gstart, glo, ghi, _blocks) in gmeta:
            for off, ln in ((0, glo), (glo, ghi)):
                if ln == 0:
                    continue
                seg = idx_slots[(gstart + off) * P:(gstart + off + ln) * P]
                wrapped = seg.reshape(-1, 16).T  # [16, n/16]
                parts.append(np.tile(wrapped, (8, 1)))
        idx_alls.append(np.concatenate(parts, axis=1))
        dstrel_alls.append(np.ascontiguousarray(dst_slots.reshape(ctot, P).T))

    # dense per-core node-value layouts
    npad = nb * P
    xa = np.zeros((n_cores, P, npad), np.float32)
    xb = np.zeros((n_cores, P, npad), np.float32)
    recips = np.zeros((n_cores, P, 2 * nb), np.float32)
    xf = np.asarray(x, np.float32)
    for c in range(n_cores):
        shard = np.zeros((npad, D), np.float32)
        shard[:npc] = xf[c * npc:(c + 1) * npc]
        s3 = shard.reshape(nb, P, D)
        xa[c] = s3.transpose(1, 0, 2).reshape(P, npad)      # [p, b*128+f]
        xb[c] = s3.transpose(2, 0, 1).reshape(P, npad)      # [f, b*128+p]
        r = np.zeros(npad, np.float32)
        r[:npc] = recip1[c * npc:(c + 1) * npc]
        r2 = r.reshape(nb, P).T                              # [p, b]
        recips[c, :, :nb] = r2
        recips[c, :, nb:] = 2.0 * r2

    wt4 = np.ascontiguousarray(
        np.asarray(W, np.float32).reshape(D, 4, P).transpose(2, 1, 0).reshape(P, 4 * P)
    )
    # wt4[f, k*128+m] = W[m, k*128+f]
    wt4 = np.zeros((P, 4 * P), np.float32)
    Wf = np.asarray(W, np.float32)
    for k in range(4):
        wt4[:, k * P:(k + 1) * P] = Wf[:, k * P:(k + 1) * P].T
    biasr = np.asarray(b, np.float32).reshape(1, P)

    x1 = np.zeros((n_cores * rows1, D), np.float32)
    x2 = np.zeros((n_cores * rows2, D), np.float32)
    for c in range(n_cores):
        x1[c * rows1:(c + 1) * rows1] = xf[c * npc:c * npc + rows1]
        x2[c * rows2:(c + 1) * rows2] = xf[c * npc + rows1:(c + 1) * npc]
    xt1 = np.ascontiguousarray(x1.astype(BF16))
    xt2 = np.ascontiguousarray(x2.astype(BF16))

    params = dict(
        N=N, npc=npc, nb=nb, ctot=ctot, n_halves=n_halves,
        split_b=split_b, rows1=rows1, rows2=rows2,
        cap=cap, lo_col=lo_col, hi_col=hi_col, gmeta=gmeta,
        idx_cols=None,
    )
    # per-call idx column offsets within idx_all ([128, ctot*8])
    idx_off = {}
    off = 0
    for gi, (gstart, glo, ghi, _blocks) in enumerate(gmeta):
        idx_off[(gi, 0)] = (off, glo)
        off += glo * 8
        idx_off[(gi, 1)] = (off, ghi)
        off += ghi * 8
    params["idx_off"] = idx_off
    params["idx_width"] = off

    per_core_inputs = []
    for c in range(n_cores):
        per_core_inputs.append({
            "xt1": xt1,
            "xt2": xt2,
            "xa": xa[c],
            "xb": xb[c],
            "idx_all": idx_alls[c],
            "dstrel": dstrel_alls[c],
            "recips": recips[c],
            "wt4": wt4,
            "biasr": biasr,
        })
    return params, per_core_inputs


# ---------------------------------------------------------------- program

def _build(params, n_cores, stage="full"):
    import concourse.bacc as bacc
    import concourse.bass as bass
    import concourse.mybir as mybir
    import concourse.tile as tile
    from concourse.masks import make_identity

    N = params["N"]
    npc, nb, ctot = params["npc"], params["nb"], params["ctot"]
    split_b, rows1, rows2 = params["split_b"], params["rows1"], params["rows2"]
    cap, lo_col, hi_col = params["cap"], params["lo_col"], params["hi_col"]
    gmeta, idx_off, idx_width = params["gmeta"], params["idx_off"], params["idx_width"]
    npad = nb * P
    f32, bf16, i16 = mybir.dt.float32, mybir.dt.bfloat16, mybir.dt.int16
    Copy = mybir.ActivationFunctionType.Copy
    Alu = mybir.AluOpType

    nc = bacc.Bacc("TRN2", target_bir_lowering=False, debug=False,
                   num_devices=n_cores, num_swdge_queues=4)

    xt1_d = nc.dram_tensor("xt1", [n_cores * rows1, P], bf16, kind="ExternalInput")
    xt2_d = nc.dram_tensor("xt2", [n_cores * rows2, P], bf16, kind="ExternalInput")
    xa_d = nc.dram_tensor("xa", [P, npad], f32, kind="ExternalInput")
    xb_d = nc.dram_tensor("xb", [P, npad], f32, kind="ExternalInput")
    idx_d = nc.dram_tensor("idx_all", [P, idx_width], i16, kind="ExternalInput")
    dstrel_d = nc.dram_tensor("dstrel", [P, ctot], i16, kind="ExternalInput")
    recips_d = nc.dram_tensor("recips", [P, 2 * nb], f32, kind="ExternalInput")
    wt4_d = nc.dram_tensor("wt4", [P, 4 * P], f32, kind="ExternalInput")
    bias_d = nc.dram_tensor("biasr", [1, P], f32, kind="ExternalInput")
    out_d = nc.dram_tensor("outT", [P, npad], f32, kind="ExternalOutput")

    cmax = max(g[1] + g[2] for g in gmeta)
    cbmax = int(cap.sum(axis=1).max())

    with tile.TileContext(nc) as tc:
        with tc.tile_pool(name="const", bufs=1) as const, \
             tc.tile_pool(name="persist", bufs=1) as persist, \
             tc.tile_pool(name="msgsp", bufs=2) as msgsp, \
             tc.tile_pool(name="mp", bufs=3) as mp, \
             tc.tile_pool(name="smallp", bufs=4) as smallp, \
             tc.tile_pool(name="pp", bufs=3, space="PSUM") as pp, \
             tc.tile_pool(name="pt", bufs=2, space="PSUM") as pt, \
             tc.tile_pool(name="pf", bufs=2, space="PSUM") as pf, \
             tc.tile_pool(name="dram", bufs=1, space="DRAM") as dram:

            idx_sb = const.tile([P, idx_width], i16)
            nc.sync.dma_start(idx_sb[:], idx_d.ap())
            dstrel_sb = const.tile([P, ctot], i16)
            nc.sync.dma_start(dstrel_sb[:], dstrel_d.ap())
            recips_sb = const.tile([P, 2 * nb], f32)
            nc.sync.dma_start(recips_sb[:], recips_d.ap())
            wt_sb = const.tile([P, 4 * P], f32)
            nc.sync.dma_start(wt_sb[:], wt4_d.ap())
            bias_sb = const.tile([1, P], f32)
            nc.sync.dma_start(bias_sb[:], bias_d.ap())
            ones_sb = const.tile([1, P], f32)
            nc.vector.memset(ones_sb[:], 1.0)
            iota_sb = const.tile([P, P], i16)
            nc.gpsimd.iota(iota_sb[:], pattern=[[1, P]], base=0,
                           channel_multiplier=0)
            ident = const.tile([P, P], f32)
            make_identity(nc, ident[:])

            ta0 = persist.tile([P, npad], f32)   # T0 shard, layout A
            nc.sync.dma_start(ta0[:], xa_d.ap())
            ta1 = persist.tile([P, npad], f32)   # T1 shard, layout A
            out_acc = persist.tile([P, npad], f32)

            # per term (round 1,2): half-tables [h0, h1] + private shard halves
            tab10 = dram.tile([n_cores * rows1, P], bf16, addr_space="Shared")
            tab11 = dram.tile([n_cores * rows2, P], bf16, addr_space="Shared")
            tab20 = dram.tile([n_cores * rows1, P], bf16, addr_space="Shared")
            tab21 = dram.tile([n_cores * rows2, P], bf16, addr_space="Shared")
            shard10 = dram.tile([rows1, P], bf16)
            shard11 = dram.tile([rows2, P], bf16)
            shard20 = dram.tile([rows1, P], bf16)
            shard21 = dram.tile([rows2, P], bf16)
            tabs = [[tab10, tab11], [tab20, tab21]]
            shards = [[shard10, shard11], [shard20, shard21]]

            # ---- init: out_acc = bias + W0^T @ x  (per block), layout B
            for b in range(nb):
                ncols = slice(b * P, (b + 1) * P)
                xb_t = smallp.tile([P, P], f32, tag="xb_t")
                nc.sync.dma_start(xb_t[:], xb_d.ap()[:, ncols])
                pf_t = pf.tile([P, P], f32)
                nc.tensor.matmul(pf_t[:], lhsT=bias_sb[:], rhs=ones_sb[:],
                                 start=True, stop=False)
                nc.tensor.matmul(pf_t[:], lhsT=wt_sb[:, 0:P], rhs=xb_t[:],
                                 start=False, stop=True)
                nc.vector.tensor_copy(out_acc[:, ncols], pf_t[:])

            # ---- rounds
            qctr = [0]  # round-robin gather calls across the 4 SWDGE queues
            rounds = {"init": (), "r1": (1,), "r1nofold": (1,), "r1noCC": (1,),
                      "r1notab": (1,), "r12": (1, 2)}.get(stage, (1, 2, 3))
            for r in rounds:
                if r == 1:
                    half_aps = [xt1_d.ap(), xt2_d.ap()]
                else:
                    half_aps = [tabs[r - 2][0][:], tabs[r - 2][1][:]]

                for gi, (gstart, glo, ghi, blocks) in enumerate(gmeta):
                    cg = glo + ghi
                    msgs = msgsp.tile([P, cmax * P], bf16, tag="msgs")
                    m3 = msgs[:].rearrange("p (c f) -> p c f", c=cmax)
                    CALL_CH = 8  # dma_gather ucode caps at 1024 idxs/call (2048 fails on HW)
                    for half, (off, ln) in ((0, idx_off[(gi, 0)]),
                                            (1, idx_off[(gi, 1)])):
                        if ln == 0:
                            continue
                        colbase = 0 if half == 0 else glo
                        for s in range(0, ln, CALL_CH):
                            w = min(CALL_CH, ln - s)
                            nc.gpsimd.dma_gather(
                                out_ap=m3[:, colbase + s:colbase + s + w, :],
                                in_ap=half_aps[half],
                                idxs_ap=idx_sb[:, off + s * 8:off + (s + w) * 8],
                                num_idxs=w * P,
                                num_idxs_reg=w * P,
                                elem_size=P,
                                elem_step=P,
                                queue_num=qctr[0] % 4,
                            )
                            qctr[0] += 1

                    for b in blocks:
                        clo, chi = int(cap[b, 0]), int(cap[b, 1])
                        cb = clo + chi
                        ncols = slice(b * P, (b + 1) * P)
                        m_t = mp.tile([P, cbmax * P], bf16, tag="m_t")
                        oh3 = m_t[:].rearrange("p (c f) -> p c f", c=cbmax)
                        i3 = iota_sb[:].rearrange("p (o f) -> p o f", o=1)
                        for (off_c, ln, gcol) in ((0, clo, lo_col[b]),
                                                  (clo, chi, hi_col[b])):
                            if ln == 0:
                                continue
                            d3 = dstrel_sb[:, gcol:gcol + ln] \
                                .rearrange("p (c o) -> p c o", o=1) \
                                .to_broadcast([P, ln, P])
                            nc.vector.tensor_tensor(
                                out=oh3[:, off_c:off_c + ln, :], in0=d3,
                                in1=i3.to_broadcast([P, ln, P]),
                                op=Alu.is_equal)
                        pp_t = pp.tile([P, P], f32)
                        for ci in range(cb):
                            if ci < clo:
                                mcol = int(lo_col[b] - gstart + ci)
                            else:
                                mcol = int(hi_col[b] - gstart + ci - clo)
                            nc.tensor.matmul(pp_t[:], lhsT=oh3[:, ci, :],
                                             rhs=m3[:, mcol, :],
                                             start=(ci == 0),
                                             stop=(ci == cb - 1))
                        # epilogue: T_new = recip*psum [- T_prev]
                        if r == 1:
                            tnew = ta1[:, ncols]
                            nc.scalar.activation(
                                out=tnew, in_=pp_t[:], func=Copy,
                                scale=recips_sb[:, b:b + 1])
                        else:
                            tmp_t = smallp.tile([P, P], f32, tag="tmp_t")
                            nc.scalar.activation(
                                out=tmp_t[:], in_=pp_t[:], func=Copy,
                                scale=recips_sb[:, nb + b:nb + b + 1])
                            tprev = (ta0 if r == 2 else ta1)[:, ncols]
                            tnew_t = smallp.tile([P, P], f32, tag="tnew_t")
                            nc.vector.tensor_tensor(
                                out=tnew_t[:], in0=tmp_t[:], in1=tprev,
                                op=Alu.subtract)
                            tnew = tnew_t[:]
                        if stage in ("nofold", "r1nofold"):
                            continue
                        # fold term r into out_acc
                        pt_t = pt.tile([P, P], f32)
                        nc.tensor.transpose(out=pt_t[:], in_=tnew, identity=ident[:])
                        ht_t = smallp.tile([P, P], f32, tag="ht_t")
                        nc.scalar.activation(out=ht_t[:], in_=pt_t[:], func=Copy)
                        pf_t = pf.tile([P, P], f32)
                        nc.tensor.matmul(pf_t[:], lhsT=wt_sb[:, r * P:(r + 1) * P],
                                         rhs=ht_t[:], start=True, stop=True)
                        nc.vector.tensor_tensor(out=out_acc[:, ncols],
                                                in0=out_acc[:, ncols],
                                                in1=pf_t[:], op=Alu.add)
                        # table shard write (rounds 1, 2)
                        if r < 3 and stage != "r1notab":
                            rows = min(npc - b * P, P)
                            tb_t = smallp.tile([P, P], bf16, tag="tb_t")
                            nc.scalar.activation(out=tb_t[:], in_=tnew, func=Copy)
                            if b < split_b:
                                sh_ap = shards[r - 1][0][b * P:b * P + rows, :]
                            else:
                                boff = (b - split_b) * P
                                sh_ap = shards[r - 1][1][boff:boff + rows, :]
                            nc.sync.dma_start(sh_ap, tb_t[:rows, :])
                            # half-0 shard complete mid-round: AllGather it now
                            # so next round's half-0 gathers start early
                            if b == split_b - 1 and stage != "r1noCC":
                                nc.gpsimd.collective_compute(
                                    "AllGather", Alu.bypass,
                                    replica_groups=[list(range(n_cores))],
                                    ins=[shards[r - 1][0].opt()],
                                    outs=[tabs[r - 1][0].opt()],
                                )
                if r < 3 and stage not in ("r1noCC", "r1notab"):
                    nc.gpsimd.collective_compute(
                        "AllGather", Alu.bypass,
                        replica_groups=[list(range(n_cores))],
                        ins=[shards[r - 1][1].opt()],
                        outs=[tabs[r - 1][1].opt()],
                    )

            nc.sync.dma_start(out_d.ap(), out_acc[:])

    nc.compile()
    return nc


# ---------------------------------------------------------------- driver

def _run(inputs, n_cores=8, trace=False, stage="full"):
    from concourse.bass_utils import run_bass_kernel_spmd

    x = np.asarray(inputs["x"])
    edge_index = np.asarray(inputs["edge_index"])
    W = np.asarray(inputs["W"])
    b = np.asarray(inputs["b"])
    N, D = x.shape
    npc = N // n_cores

    params, in_maps = _prep(x, edge_index, W, b, n_cores)
    nc = _build(params, n_cores, stage=stage)
    res = run_bass_kernel_spmd(nc, in_maps, list(range(n_cores)), trace=trace)

    nb = params["nb"]
    out = np.empty((N, D), np.float32)
    for c in range(n_cores):
        o = res.results[c]["outT"]                       # [128, nb*128]
        o3 = o.reshape(P, nb, P).transpose(1, 2, 0)      # [b, p, dout]
        out[c * npc:(c + 1) * npc] = o3.reshape(nb * P, D)[:npc]
    return out, res


def kernel(**inputs):
    out, _ = _run(inputs)
    return out

